# revision 38
# baseline (speedup 1.0000x reference)
"""Trainium2 Bass kernel for nn_AssignAttention (hard-assignment MoE-routing attention).

Math (forward): for each (b, h, key-token s), the key token is hard-assigned to
group n* = argmax_n (q_bhn . k_bhs); output per group = sum of assigned v vectors
scaled by 1/(count+1), then projected.  The straight-through softmax terms cancel
in forward up to ~1e-7, so only the argmax routing matters.

Strategy:
 - Pure data-parallel over batch B=16 across 8 cores (2 batches/core), no collectives.
 - Host precomputes t[b,h,n,:] = Wk_h^T Wq_h query[b,n] so attention logits are
   attn[s, (h,n)] = key[b,s,:] . t[b,h,n,:]  -- one C-contraction against raw key.
 - Host pre-transposes key to keyT [C, S] so the C-contraction needs no on-device
   transposes; key is read exactly once from HBM (memory roofline).
 - Attention logits use float32r matmuls (1 cyc/row, ~13-bit mantissa): measured
   argmax flip-induced error ~0.008 rel, well within tolerance. v/output paths in
   float32r/bf16.
 - Per 128-row s-subtile: argmax over each head's 64 logit columns (free-axis
   reduce_max + one broadcast is_equal -> bf16 one-hot), then PSUM-accumulate
   head-PAIR-packed o += aT_pair^T @ [v|1|v|1] (128-col bf16 lhsT enables fast
   weight loads; the ones column yields per-group counts).  The o-matmuls of
   subtile i are issued after subtile i+1's attn/v matmuls so the PE never
   head-of-line blocks on the DVE one-hot.  Epilogue scales by 1/(cnt+1),
   transposes, and applies the output projection; bias is added on host.
 - PE warmup matmuls fill the initial DMA wait so the HAM clock-gate opens early.
"""
import sys

sys.path.insert(0, "/opt/trn_rl_repo")

import numpy as np
import ml_dtypes

import concourse.bass as bass
import concourse.mybir as mybir
import concourse.tile as tile
from concourse.bass_utils import run_bass_kernel_spmd
from concourse.masks import make_identity

B, N, S, C, H = 16, 64, 4096, 384, 6
DH = C // H  # 64
NCORES = 8
BPC = B // NCORES  # batches per core = 2
CT = C // 128  # c-tiles = 3
S_CHUNK = 512
N_CHUNKS = S // S_CHUNK  # 8
SUBS = S_CHUNK // 128  # 4 s-subtiles per chunk

F32 = mybir.dt.float32
F32R = mybir.dt.float32r
BF16 = mybir.dt.bfloat16

LAST_RESULT = None  # stash of BassKernelResults for profiling in test.py


def _split_multiwaits(nc):
    """walrus codegen in this toolchain accepts at most one sync-wait per
    instruction; hoist extras onto standalone wait-only EventSemaphore
    instructions placed immediately before (same engine, so ordering holds)."""
    for fn in nc.m.functions:
        for blk in fn.blocks:
            new = []
            for inst in blk.instructions:
                si = inst.sync_info
                if si is not None and si.on_wait and len(si.on_wait) > 1:
                    for w in si.on_wait[:-1]:
                        ev = mybir.InstEventSemaphore(
                            name=nc.get_next_instruction_name(), ins=[], outs=[]
                        )
                        ev.engine = inst.engine
                        ev.sync_info = mybir.SyncInfo(on_wait=[w], on_update=[])
                        new.append(ev)
                    inst.sync_info = mybir.SyncInfo(
                        on_wait=[si.on_wait[-1]], on_update=si.on_update
                    )
                new.append(inst)
            blk.instructions = new


def _build_kernel():
    nc = bass.Bass()
    keyT_d = nc.declare_dram_parameter("keyT", [BPC, C, S], F32R, isOutput=False)
    tc_d = nc.declare_dram_parameter("tc", [BPC, C, C], F32R, isOutput=False)
    wvt_d = nc.declare_dram_parameter("wvt", [C, C], F32R, isOutput=False)
    wpt_d = nc.declare_dram_parameter("wpt", [C, C], BF16, isOutput=False)
    out_d = nc.declare_dram_parameter("out", [BPC, N, C], F32, isOutput=True)

    with tile.TileContext(nc) as tc:
        with (
            tc.tile_pool(name="consts", bufs=1) as consts,
            tc.tile_pool(name="perb", bufs=2) as perb,
            tc.tile_pool(name="keyp", bufs=4) as keyp,
            tc.tile_pool(name="work", bufs=4) as work,
            tc.tile_pool(name="epi", bufs=2) as epi,
            tc.tile_pool(name="ps_attn", bufs=3, space="PSUM") as ps_attn,
            tc.tile_pool(name="ps_v", bufs=2, space="PSUM") as ps_v,
            tc.tile_pool(name="ps_o", bufs=1, space="PSUM") as ps_o,
            tc.tile_pool(name="ps_epi", bufs=1, space="PSUM") as ps_epi,
        ):
            # issue the first keyT chunk DMA before everything else so the PE
            # can start as soon as possible
            kt_first = keyp.tile([128, CT, S_CHUNK], F32R, tag="kt")
            nc.sync.dma_start(
                out=kt_first[:],
                in_=keyT_d[0].rearrange("(ct p) s -> p ct s", p=128)[:, :, 0:S_CHUNK],
            )
            tc_first = perb.tile([128, CT, C], F32R, tag="tc_sb")
            nc.sync.dma_start(
                out=tc_first[:],
                in_=tc_d[0].rearrange("(ct p) hn -> p ct hn", p=128),
            )
            # global constants
            wvt_sb = consts.tile([128, CT, C], F32R)  # [c_in_p, ct, c_out]
            nc.sync.dma_start(
                out=wvt_sb[:], in_=wvt_d.rearrange("(ct p) co -> p ct co", p=128)
            )
            wpt_sb = consts.tile([128, CT, C], BF16)  # [hd_p, ct, c_out]
            nc.sync.dma_start(
                out=wpt_sb[:], in_=wpt_d.rearrange("(ct p) co -> p ct co", p=128)
            )
            # two stacked 64x64 identities so transposes of partition-offset-64
            # slices have a matching-base-partition rhs
            ident2 = consts.tile([128, N], BF16)
            make_identity(nc, ident2[0:N, :])
            make_identity(nc, ident2[N : 2 * N, :])

            # PE warmup: back-to-back matmuls on scratch during the initial DMA
            # wait, so the HAM clock-gate reaches 8/8 before real work arrives.
            # Uninitialized data is fine: the psum bank is never read and its
            # reuse starts with start=True (overwrite).
            warm_sb = consts.tile([128, 640], BF16)
            nc.gpsimd.memset(warm_sb[:], 0.0)
            warm_ps = ps_attn.tile([128, 512], F32, tag="attn_ps")
            for _ in range(8):
                nc.tensor.matmul(
                    warm_ps[:, 0:256], warm_sb[:, 0:128], warm_sb[:, 128:384],
                    start=True, stop=True,
                )

            for b in range(BPC):
                if b == 0:
                    tc_sb = tc_first
                else:
                    tc_sb = perb.tile([128, CT, C], F32R, tag="tc_sb")
                    nc.sync.dma_start(
                        out=tc_sb[:],
                        in_=tc_d[b].rearrange("(ct p) hn -> p ct hn", p=128),
                    )
                # per-group accumulator, head-PAIR packed: for pair p, partition
                # rows 0..63 = head 2p groups, rows 64..127 = head 2p+1 groups;
                # col 64 = counts for both heads; cols 0..63 / 65..128 hold the
                # two heads' v-sums (off-diagonal blocks are junk, never read).
                # Zeroed explicitly; the accumulating matmuls use start=False so
                # their order doesn't matter (add-or-overwrite onto zeros commutes).
                o_ps = ps_o.tile([128, CT, 2 * DH + 2], F32)
                nc.vector.memset(o_ps[:], 0.0)

                keyT_b = keyT_d[b].rearrange("(ct p) s -> p ct s", p=128)
                # software pipeline: issue subtile i's o-matmuls AFTER subtile
                # i+1's attn/v matmuls, so the PE never head-of-line-blocks on
                # DVE's one-hot for the current subtile.
                pending = None  # (aT, v65) of previous subtile

                def flush_o(stop):
                    aT_p, v65_p = pending
                    for p in range(CT):
                        nc.tensor.matmul(
                            o_ps[:, p, :],
                            aT_p[:].rearrange("q h n -> q (h n)")[:, 2 * p * N : (2 * p + 2) * N],
                            v65_p[:].rearrange("q h d -> q (h d)")[
                                :, 2 * p * (DH + 1) : (2 * p + 2) * (DH + 1)
                            ],
                            start=False,
                            stop=stop,
                            skip_group_check=True,
                        )

                for ch in range(N_CHUNKS):
                    if b == 0 and ch == 0:
                        kt_sb = kt_first
                    else:
                        kt_sb = keyp.tile([128, CT, S_CHUNK], F32R, tag="kt")
                        nc.sync.dma_start(
                            out=kt_sb[:],
                            in_=keyT_b[:, :, ch * S_CHUNK : (ch + 1) * S_CHUNK],
                        )
                    for sub in range(SUBS):
                        sl = slice(sub * 128, (sub + 1) * 128)
                        attn_ps = ps_attn.tile([128, C], F32)
                        v_ps = ps_v.tile([128, C], F32)
                        # all attn matmuls first so the logit group closes
                        # ~3 matmuls earlier and DVE's argmax starts sooner
                        for ct in range(CT):
                            nc.tensor.matmul(
                                attn_ps[:],
                                kt_sb[:, ct, sl],
                                tc_sb[:, ct, :],
                                start=(ct == 0),
                                stop=(ct == CT - 1),
                            )
                        for ct in range(CT):
                            nc.tensor.matmul(
                                v_ps[:],
                                kt_sb[:, ct, sl],
                                wvt_sb[:, ct, :],
                                start=(ct == 0),
                                stop=(ct == CT - 1),
                            )
                        if pending is not None:
                            flush_o(stop=False)
                        # per-head argmax -> one-hot (bf16)
                        gmax = work.tile([128, H], F32)
                        nc.vector.reduce_max(
                            out=gmax[:],
                            in_=attn_ps[:].rearrange("p (h n) -> p h n", h=H),
                            axis=mybir.AxisListType.X,
                        )
                        aT = work.tile([128, H, N], BF16)
                        g = gmax[:]
                        g_bcast = bass.AP(
                            tensor=g.tensor, offset=g.offset,
                            ap=[g.ap[0], g.ap[1], [0, N]],
                        )
                        nc.vector.tensor_tensor(
                            out=aT[:],
                            in0=attn_ps[:].rearrange("p (h n) -> p h n", h=H),
                            in1=g_bcast,
                            op=mybir.AluOpType.is_equal,
                        )
                        # v (bf16) with a ones-column per head for counts
                        v65 = work.tile([128, H, DH + 1], BF16)
                        nc.scalar.copy(
                            out=v65[:, :, 0:DH],
                            in_=v_ps[:].rearrange("p (h d) -> p h d", h=H),
                        )
                        nc.gpsimd.memset(v65[:, :, DH : DH + 1], 1.0)
                        pending = (aT, v65)
                flush_o(stop=True)
                pending = None
                # epilogue for this b: copy the accumulator to SBUF once so the
                # PSUM bank frees immediately for the next batch, then scale by
                # 1/(cnt+1) (cnt in col 64 for both heads of each pair) and
                # transpose to [hd, n]
                o_sb = epi.tile([128, CT, 2 * DH + 2], F32)
                nc.scalar.copy(out=o_sb[:], in_=o_ps[:])
                scl = epi.tile([128, CT], F32)
                nc.vector.tensor_scalar(
                    out=scl[:],
                    in0=o_sb[:, :, DH],
                    scalar1=1.0,
                    scalar2=None,
                    op0=mybir.AluOpType.add,
                )
                nc.vector.reciprocal(out=scl[:], in_=scl[:])
                osc = epi.tile([128, CT, DH], BF16)
                for p in range(CT):
                    nc.vector.tensor_scalar(
                        out=osc[0:N, p, :],
                        in0=o_sb[0:N, p, 0:DH],
                        scalar1=scl[0:N, p : p + 1],
                        scalar2=None,
                        op0=mybir.AluOpType.mult,
                    )
                    nc.vector.tensor_scalar(
                        out=osc[N : 2 * N, p, :],
                        in0=o_sb[N : 2 * N, p, DH + 1 : 2 * DH + 1],
                        scalar1=scl[N : 2 * N, p : p + 1],
                        scalar2=None,
                        op0=mybir.AluOpType.mult,
                    )
                # osc[0:64, p, :] = [n, dh] of head 2p -> oT rows 128p+dh;
                # osc[64:128, p, :] = [n, dh] of head 2p+1 -> oT rows 128p+64+dh
                oT_ps = ps_epi.tile([128, CT, N], BF16)
                for p in range(CT):
                    nc.tensor.transpose(
                        oT_ps[0:N, p, :], osc[0:N, p, :], ident2[0:N, :]
                    )
                    nc.tensor.transpose(
                        oT_ps[N : 2 * N, p, :],
                        osc[N : 2 * N, p, :],
                        ident2[N : 2 * N, :],
                    )
                oT_sb = epi.tile([128, CT, N], BF16)
                nc.scalar.copy(out=oT_sb[:], in_=oT_ps[:])
                out_ps = ps_epi.tile([N, C], F32)
                for ct in range(CT):
                    nc.tensor.matmul(
                        out_ps[:],
                        oT_sb[:, ct, :],
                        wpt_sb[:, ct, :],
                        start=(ct == 0),
                        stop=(ct == CT - 1),
                    )
                out_sb = epi.tile([N, C], F32)
                nc.scalar.copy(out=out_sb[:], in_=out_ps[:])
                nc.sync.dma_start(out=out_d[b], in_=out_sb[:])

    _split_multiwaits(nc)
    return nc


_NC_CACHE = None


def _get_nc():
    global _NC_CACHE
    if _NC_CACHE is None:
        _NC_CACHE = _build_kernel()
    return _NC_CACHE


def kernel(query, key, Wq, Wk, Wv, Wp, bp):
    global LAST_RESULT
    query = np.ascontiguousarray(query, dtype=np.float32)
    key = np.ascontiguousarray(key, dtype=np.float32)
    Wq = np.asarray(Wq, dtype=np.float32)
    Wk = np.asarray(Wk, dtype=np.float32)
    Wv = np.asarray(Wv, dtype=np.float32)
    Wp = np.asarray(Wp, dtype=np.float32)
    bp = np.asarray(bp, dtype=np.float32)

    # host prep: t[b,h,n,:] = Wk_h^T Wq_h query[b,n]  (tiny; never touches `key`)
    q = query @ Wq.T  # [B, N, C]
    qh = q.reshape(B, N, H, DH).transpose(0, 2, 1, 3)  # [B,H,N,DH]
    Wk_h = Wk.reshape(H, DH, C)
    t = np.einsum("bhnd,hdc->bhnc", qh, Wk_h)  # [B,H,N,C]
    # Tc[b] layout: [C, (h n)] with column h*N+n = t[b,h,n,:]
    Tc = np.ascontiguousarray(
        t.transpose(0, 3, 1, 2).reshape(B, C, H * N), dtype=np.float32
    )
    keyT = np.ascontiguousarray(key.transpose(0, 2, 1), dtype=np.float32)  # [B,C,S]
    WvT = np.ascontiguousarray(Wv.T, dtype=np.float32)
    WpT = np.ascontiguousarray(Wp.T).astype(ml_dtypes.bfloat16)

    nc = _get_nc()
    in_maps = [
        {
            "keyT": keyT[i * BPC : (i + 1) * BPC],
            "tc": Tc[i * BPC : (i + 1) * BPC],
            "wvt": WvT,
            "wpt": WpT,
        }
        for i in range(NCORES)
    ]
    try:
        res = run_bass_kernel_spmd(nc, in_maps, core_ids=list(range(NCORES)))
    except Exception:
        # transient NRT device errors have been observed; retry once
        res = run_bass_kernel_spmd(nc, in_maps, core_ids=list(range(NCORES)))
    LAST_RESULT = res
    out = np.concatenate([res.results[i]["out"] for i in range(NCORES)], axis=0)
    return (out + bp).astype(np.float32)


# revision 39
# speedup vs baseline: 1.0217x; 1.0217x over previous
"""Trainium2 Bass kernel for nn_AssignAttention (hard-assignment MoE-routing attention).

Math (forward): for each (b, h, key-token s), the key token is hard-assigned to
group n* = argmax_n (q_bhn . k_bhs); output per group = sum of assigned v vectors
scaled by 1/(count+1), then projected.  The straight-through softmax terms cancel
in forward up to ~1e-7, so only the argmax routing matters.

Strategy:
 - Pure data-parallel over batch B=16 across 8 cores (2 batches/core), no collectives.
 - Host precomputes t[b,h,n,:] = Wk_h^T Wq_h query[b,n] so attention logits are
   attn[s, (h,n)] = key[b,s,:] . t[b,h,n,:]  -- one C-contraction against raw key.
 - Host pre-transposes key to keyT [C, S] so the C-contraction needs no on-device
   transposes; key is read exactly once from HBM (memory roofline).
 - Attention logits use float32r matmuls (1 cyc/row, ~13-bit mantissa): measured
   argmax flip-induced error ~0.008 rel, well within tolerance. v/output paths in
   float32r/bf16.
 - Per 128-row s-subtile: argmax over each head's 64 logit columns (free-axis
   reduce_max + one broadcast is_equal -> bf16 one-hot), then PSUM-accumulate
   head-PAIR-packed o += aT_pair^T @ [v|1|v|1] (128-col bf16 lhsT enables fast
   weight loads; the ones column yields per-group counts).  The o-matmuls of
   subtile i are issued after subtile i+1's attn/v matmuls so the PE never
   head-of-line blocks on the DVE one-hot.  Epilogue scales by 1/(cnt+1),
   transposes, and applies the output projection; bias is added on host.
 - PE warmup matmuls fill the initial DMA wait so the HAM clock-gate opens early.
"""
import sys

sys.path.insert(0, "/opt/trn_rl_repo")

import numpy as np
import ml_dtypes

import concourse.bass as bass
import concourse.mybir as mybir
import concourse.tile as tile
from concourse.bass_utils import run_bass_kernel_spmd
from concourse.masks import make_identity

B, N, S, C, H = 16, 64, 4096, 384, 6
DH = C // H  # 64
NCORES = 8
BPC = B // NCORES  # batches per core = 2
CT = C // 128  # c-tiles = 3
S_CHUNK = 512
N_CHUNKS = S // S_CHUNK  # 8
SUBS = S_CHUNK // 128  # 4 s-subtiles per chunk

F32 = mybir.dt.float32
F32R = mybir.dt.float32r
BF16 = mybir.dt.bfloat16

LAST_RESULT = None  # stash of BassKernelResults for profiling in test.py


def _split_multiwaits(nc):
    """walrus codegen in this toolchain accepts at most one sync-wait per
    instruction; hoist extras onto standalone wait-only EventSemaphore
    instructions placed immediately before (same engine, so ordering holds)."""
    for fn in nc.m.functions:
        for blk in fn.blocks:
            new = []
            for inst in blk.instructions:
                si = inst.sync_info
                if si is not None and si.on_wait and len(si.on_wait) > 1:
                    for w in si.on_wait[:-1]:
                        ev = mybir.InstEventSemaphore(
                            name=nc.get_next_instruction_name(), ins=[], outs=[]
                        )
                        ev.engine = inst.engine
                        ev.sync_info = mybir.SyncInfo(on_wait=[w], on_update=[])
                        new.append(ev)
                    inst.sync_info = mybir.SyncInfo(
                        on_wait=[si.on_wait[-1]], on_update=si.on_update
                    )
                new.append(inst)
            blk.instructions = new


def _build_kernel():
    nc = bass.Bass()
    keyT_d = nc.declare_dram_parameter("keyT", [BPC, C, S], F32R, isOutput=False)
    tc_d = nc.declare_dram_parameter("tc", [BPC, C, C], F32R, isOutput=False)
    wvt_d = nc.declare_dram_parameter("wvt", [C, C], F32R, isOutput=False)
    wpt_d = nc.declare_dram_parameter("wpt", [C, C], BF16, isOutput=False)
    out_d = nc.declare_dram_parameter("out", [BPC, N, C], F32, isOutput=True)

    with tile.TileContext(nc) as tc:
        with (
            tc.tile_pool(name="consts", bufs=1) as consts,
            tc.tile_pool(name="perb", bufs=2) as perb,
            tc.tile_pool(name="keyp", bufs=4) as keyp,
            tc.tile_pool(name="work", bufs=4) as work,
            tc.tile_pool(name="epi", bufs=2) as epi,
            tc.tile_pool(name="ps_attn", bufs=3, space="PSUM") as ps_attn,
            tc.tile_pool(name="ps_v", bufs=2, space="PSUM") as ps_v,
            tc.tile_pool(name="ps_o", bufs=1, space="PSUM") as ps_o,
            tc.tile_pool(name="ps_epi", bufs=1, space="PSUM") as ps_epi,
        ):
            # issue the first keyT chunk DMA before everything else so the PE
            # can start as soon as possible
            kt_first = keyp.tile([128, CT, S_CHUNK], F32R, tag="kt")
            nc.sync.dma_start(
                out=kt_first[:],
                in_=keyT_d[0].rearrange("(ct p) s -> p ct s", p=128)[:, :, 0:S_CHUNK],
            )
            tc_first = perb.tile([128, CT, C], F32R, tag="tc_sb")
            nc.sync.dma_start(
                out=tc_first[:],
                in_=tc_d[0].rearrange("(ct p) hn -> p ct hn", p=128),
            )
            # global constants
            wvt_sb = consts.tile([128, CT, C], F32R)  # [c_in_p, ct, c_out]
            nc.sync.dma_start(
                out=wvt_sb[:], in_=wvt_d.rearrange("(ct p) co -> p ct co", p=128)
            )
            wpt_sb = consts.tile([128, CT, C], BF16)  # [hd_p, ct, c_out]
            nc.sync.dma_start(
                out=wpt_sb[:], in_=wpt_d.rearrange("(ct p) co -> p ct co", p=128)
            )
            # two stacked 64x64 identities so transposes of partition-offset-64
            # slices have a matching-base-partition rhs
            ident2 = consts.tile([128, N], BF16)
            make_identity(nc, ident2[0:N, :])
            make_identity(nc, ident2[N : 2 * N, :])

            # PE warmup: back-to-back matmuls on scratch during the initial DMA
            # wait, so the HAM clock-gate reaches 8/8 before real work arrives.
            # Uninitialized data is fine: the psum bank is never read and its
            # reuse starts with start=True (overwrite).
            warm_sb = consts.tile([128, 640], BF16)
            nc.gpsimd.memset(warm_sb[:], 0.0)
            warm_ps = ps_attn.tile([128, 512], F32, tag="attn_ps")
            for _ in range(8):
                nc.tensor.matmul(
                    warm_ps[:], warm_sb[:, 0:128], warm_sb[:, 128:640],
                    start=True, stop=True,
                )

            for b in range(BPC):
                if b == 0:
                    tc_sb = tc_first
                else:
                    tc_sb = perb.tile([128, CT, C], F32R, tag="tc_sb")
                    nc.sync.dma_start(
                        out=tc_sb[:],
                        in_=tc_d[b].rearrange("(ct p) hn -> p ct hn", p=128),
                    )
                # per-group accumulator, head-PAIR packed: for pair p, partition
                # rows 0..63 = head 2p groups, rows 64..127 = head 2p+1 groups;
                # col 64 = counts for both heads; cols 0..63 / 65..128 hold the
                # two heads' v-sums (off-diagonal blocks are junk, never read).
                # Zeroed explicitly; the accumulating matmuls use start=False so
                # their order doesn't matter (add-or-overwrite onto zeros commutes).
                o_ps = ps_o.tile([128, CT, 2 * DH + 2], F32)
                nc.vector.memset(o_ps[:], 0.0)

                keyT_b = keyT_d[b].rearrange("(ct p) s -> p ct s", p=128)
                # software pipeline: issue subtile i's o-matmuls AFTER subtile
                # i+1's attn/v matmuls, so the PE never head-of-line-blocks on
                # DVE's one-hot for the current subtile.
                pending = None  # (aT, v65) of previous subtile

                def flush_o(stop):
                    aT_p, v65_p = pending
                    for p in range(CT):
                        nc.tensor.matmul(
                            o_ps[:, p, :],
                            aT_p[:].rearrange("q h n -> q (h n)")[:, 2 * p * N : (2 * p + 2) * N],
                            v65_p[:].rearrange("q h d -> q (h d)")[
                                :, 2 * p * (DH + 1) : (2 * p + 2) * (DH + 1)
                            ],
                            start=False,
                            stop=stop,
                            skip_group_check=True,
                        )

                for ch in range(N_CHUNKS):
                    if b == 0 and ch == 0:
                        kt_sb = kt_first
                    else:
                        kt_sb = keyp.tile([128, CT, S_CHUNK], F32R, tag="kt")
                        nc.sync.dma_start(
                            out=kt_sb[:],
                            in_=keyT_b[:, :, ch * S_CHUNK : (ch + 1) * S_CHUNK],
                        )
                    for sub in range(SUBS):
                        sl = slice(sub * 128, (sub + 1) * 128)
                        attn_ps = ps_attn.tile([128, C], F32)
                        v_ps = ps_v.tile([128, C], F32)
                        # all attn matmuls first so the logit group closes
                        # ~3 matmuls earlier and DVE's argmax starts sooner
                        for ct in range(CT):
                            nc.tensor.matmul(
                                attn_ps[:],
                                kt_sb[:, ct, sl],
                                tc_sb[:, ct, :],
                                start=(ct == 0),
                                stop=(ct == CT - 1),
                            )
                        for ct in range(CT):
                            nc.tensor.matmul(
                                v_ps[:],
                                kt_sb[:, ct, sl],
                                wvt_sb[:, ct, :],
                                start=(ct == 0),
                                stop=(ct == CT - 1),
                            )
                        if pending is not None:
                            flush_o(stop=False)
                        # per-head argmax -> one-hot (bf16)
                        gmax = work.tile([128, H], F32)
                        nc.vector.reduce_max(
                            out=gmax[:],
                            in_=attn_ps[:].rearrange("p (h n) -> p h n", h=H),
                            axis=mybir.AxisListType.X,
                        )
                        aT = work.tile([128, H, N], BF16)
                        g = gmax[:]
                        g_bcast = bass.AP(
                            tensor=g.tensor, offset=g.offset,
                            ap=[g.ap[0], g.ap[1], [0, N]],
                        )
                        nc.vector.tensor_tensor(
                            out=aT[:],
                            in0=attn_ps[:].rearrange("p (h n) -> p h n", h=H),
                            in1=g_bcast,
                            op=mybir.AluOpType.is_equal,
                        )
                        # v (bf16) with a ones-column per head for counts
                        v65 = work.tile([128, H, DH + 1], BF16)
                        nc.scalar.copy(
                            out=v65[:, :, 0:DH],
                            in_=v_ps[:].rearrange("p (h d) -> p h d", h=H),
                        )
                        nc.gpsimd.memset(v65[:, :, DH : DH + 1], 1.0)
                        pending = (aT, v65)
                flush_o(stop=True)
                pending = None
                # epilogue for this b: copy the accumulator to SBUF once so the
                # PSUM bank frees immediately for the next batch, then scale by
                # 1/(cnt+1) (cnt in col 64 for both heads of each pair) and
                # transpose to [hd, n]
                o_sb = epi.tile([128, CT, 2 * DH + 2], F32)
                nc.scalar.copy(out=o_sb[:], in_=o_ps[:])
                scl = epi.tile([128, CT], F32)
                nc.vector.tensor_scalar(
                    out=scl[:],
                    in0=o_sb[:, :, DH],
                    scalar1=1.0,
                    scalar2=None,
                    op0=mybir.AluOpType.add,
                )
                nc.vector.reciprocal(out=scl[:], in_=scl[:])
                osc = epi.tile([128, CT, DH], BF16)
                for p in range(CT):
                    nc.vector.tensor_scalar(
                        out=osc[0:N, p, :],
                        in0=o_sb[0:N, p, 0:DH],
                        scalar1=scl[0:N, p : p + 1],
                        scalar2=None,
                        op0=mybir.AluOpType.mult,
                    )
                    nc.vector.tensor_scalar(
                        out=osc[N : 2 * N, p, :],
                        in0=o_sb[N : 2 * N, p, DH + 1 : 2 * DH + 1],
                        scalar1=scl[N : 2 * N, p : p + 1],
                        scalar2=None,
                        op0=mybir.AluOpType.mult,
                    )
                # osc[0:64, p, :] = [n, dh] of head 2p -> oT rows 128p+dh;
                # osc[64:128, p, :] = [n, dh] of head 2p+1 -> oT rows 128p+64+dh
                oT_ps = ps_epi.tile([128, CT, N], BF16)
                for p in range(CT):
                    nc.tensor.transpose(
                        oT_ps[0:N, p, :], osc[0:N, p, :], ident2[0:N, :]
                    )
                    nc.tensor.transpose(
                        oT_ps[N : 2 * N, p, :],
                        osc[N : 2 * N, p, :],
                        ident2[N : 2 * N, :],
                    )
                oT_sb = epi.tile([128, CT, N], BF16)
                nc.scalar.copy(out=oT_sb[:], in_=oT_ps[:])
                out_ps = ps_epi.tile([N, C], F32)
                for ct in range(CT):
                    nc.tensor.matmul(
                        out_ps[:],
                        oT_sb[:, ct, :],
                        wpt_sb[:, ct, :],
                        start=(ct == 0),
                        stop=(ct == CT - 1),
                    )
                out_sb = epi.tile([N, C], F32)
                nc.scalar.copy(out=out_sb[:], in_=out_ps[:])
                nc.sync.dma_start(out=out_d[b], in_=out_sb[:])

    _split_multiwaits(nc)
    return nc


_NC_CACHE = None


def _get_nc():
    global _NC_CACHE
    if _NC_CACHE is None:
        _NC_CACHE = _build_kernel()
    return _NC_CACHE


def kernel(query, key, Wq, Wk, Wv, Wp, bp):
    global LAST_RESULT
    query = np.ascontiguousarray(query, dtype=np.float32)
    key = np.ascontiguousarray(key, dtype=np.float32)
    Wq = np.asarray(Wq, dtype=np.float32)
    Wk = np.asarray(Wk, dtype=np.float32)
    Wv = np.asarray(Wv, dtype=np.float32)
    Wp = np.asarray(Wp, dtype=np.float32)
    bp = np.asarray(bp, dtype=np.float32)

    # host prep: t[b,h,n,:] = Wk_h^T Wq_h query[b,n]  (tiny; never touches `key`)
    q = query @ Wq.T  # [B, N, C]
    qh = q.reshape(B, N, H, DH).transpose(0, 2, 1, 3)  # [B,H,N,DH]
    Wk_h = Wk.reshape(H, DH, C)
    t = np.einsum("bhnd,hdc->bhnc", qh, Wk_h)  # [B,H,N,C]
    # Tc[b] layout: [C, (h n)] with column h*N+n = t[b,h,n,:]
    Tc = np.ascontiguousarray(
        t.transpose(0, 3, 1, 2).reshape(B, C, H * N), dtype=np.float32
    )
    keyT = np.ascontiguousarray(key.transpose(0, 2, 1), dtype=np.float32)  # [B,C,S]
    WvT = np.ascontiguousarray(Wv.T, dtype=np.float32)
    WpT = np.ascontiguousarray(Wp.T).astype(ml_dtypes.bfloat16)

    nc = _get_nc()
    in_maps = [
        {
            "keyT": keyT[i * BPC : (i + 1) * BPC],
            "tc": Tc[i * BPC : (i + 1) * BPC],
            "wvt": WvT,
            "wpt": WpT,
        }
        for i in range(NCORES)
    ]
    try:
        res = run_bass_kernel_spmd(nc, in_maps, core_ids=list(range(NCORES)))
    except Exception:
        # transient NRT device errors have been observed; retry once
        res = run_bass_kernel_spmd(nc, in_maps, core_ids=list(range(NCORES)))
    LAST_RESULT = res
    out = np.concatenate([res.results[i]["out"] for i in range(NCORES)], axis=0)
    return (out + bp).astype(np.float32)
